# revision 14
# baseline (speedup 1.0000x reference)
"""Trainium2 Bass kernel for MultiHeadAttention (B=4, S=2048, D=1024, H=16).

Sharding: 8 cores = 4 batches x 2 sequence-halves, no collectives. Each
core computes full k/v projections for its batch and q/attention/
out-proj/LayerNorm for its half of the sequence. A host-side column
roll of x^T makes the program identical across cores (softmax over
keys is permutation-invariant, so the k/v column order doesn't
matter): the core's q rows are always columns [0, SQ) of its x^T.

Device program (per core), all matmuls bf16 with fp32 PSUM accumulate:
  qT = Wq @ xT[:, :SQ] + bq   [D, SQ]   (d on partitions)
  kT = Wk @ xT + bk           [D, S]
  v  = x @ Wv.T + bv          [S, D]    (s on partitions), augmented
                                        with a ones column per head
  attention per head-pair m (heads 2m / 2m+1 live on partition halves
  0:64 / 64:128 of kT/qT tile m), per q-chunk n:
    scT[j]   = kT_h[:, j-chunk] . qT_h     both heads -> one 2-bank
                                           PSUM tile [128, 2, 512]
    eT[j]    = exp(scT[j] / 8)             one wide ScalarE op, bf16
    av_h[t] += eT_h[j][:, t-chunk].T @ v_aug_h[j]
        "AV-swap": eT is the stationary operand, v_aug (64 cols + ones
        column) moves, so each matmul streams only 65 columns instead
        of 512 -> half the PE time of the classic v.T @ e orientation.
        Accumulators for one head live in one PSUM bank as [4, 65]
        slices of a [128, 4, 128] tile; a full-bank zero matmul opens
        each round (clears the has_written bits + orders the start),
        then all real matmuls accumulate with start=False.
    ao_h[t]  = av_h[t][:, 0:64] * (1/av_h[t][:, 64])   per-partition
                                           scalar -> one tensor_scalar
    aot      = PE-transpose(ao)            [q, d] -> [d, q] via the
                                           identity matmul, per 128x128
  out = aoT.T @ Wo.T + bo; LayerNorm -> DRAM.

Scheduling: weights stream per head-pair so attention on pair m starts
as soon as q/k(m) and the first v d-chunk exist; the remaining
projections fill PE slack underneath the ScalarE-bound attention.
"""

import os
import sys
from contextlib import ExitStack

for _p in ("/opt/trn_rl_repo", "/root/.axon_site/_ro/trn_rl_repo"):
    if _p not in sys.path and os.path.isdir(_p):
        sys.path.insert(0, _p)

# The kernel executes through the axon jax platform; a cpu-pinned
# JAX_PLATFORMS (used for running references) would hide the NeuronCores.
# Only safe to fix before jax is first imported.
if "jax" not in sys.modules and "axon" not in os.environ.get(
        "JAX_PLATFORMS", "axon"):
    os.environ.pop("JAX_PLATFORMS")

import ml_dtypes
import numpy as np

import concourse.bacc as bacc
import concourse.mybir as mybir
import concourse.tile as tile
from concourse import library_config
from concourse.bass_utils import run_bass_kernel_spmd

BF16 = mybir.dt.bfloat16
F32 = mybir.dt.float32
AF = mybir.ActivationFunctionType
ALU = mybir.AluOpType

HD = 64  # head dim


def build_bass(S, SQ, D, H, dtype=BF16):
    """Build the per-core Bass program. S: kv seq len, SQ: q rows handled
    by this core, D: embed dim, H: total heads."""
    assert D == H * HD
    P = 128
    ET = D // P           # e (contraction) tiles; also head-pair count
    QC = min(512, SQ)     # q free-dim chunk
    QN = SQ // QC
    KC = min(512, S)      # k-proj free-dim chunk
    KN = S // KC
    VC = min(512, D)      # v/out-proj d chunk
    VN = D // VC
    HPC = VC // HD        # heads per v chunk
    MPC = HPC // 2        # head-pairs per v chunk
    SJ = S // P           # key tiles
    TQ = SQ // P          # q row tiles

    nc = bacc.Bacc("TRN2", debug=False)

    xT = nc.dram_tensor("xT", [D, S], dtype, kind="ExternalInput").ap()
    ident = nc.dram_tensor("ident", [P, P], dtype, kind="ExternalInput").ap()
    ws = {}
    for w in ("wq", "wk"):  # host-packed per head-pair: [m, p, t*d]
        ws[w] = nc.dram_tensor(w, [ET, P, ET * P], dtype,
                               kind="ExternalInput").ap()
    for w in ("wv", "wo"):
        ws[w] = nc.dram_tensor(w, [D, D], dtype, kind="ExternalInput").ap()
    bs = {
        b: nc.dram_tensor(b, [D], F32, kind="ExternalInput").ap()
        for b in ("bq", "bk", "bv", "bo", "lnw", "lnb")
    }
    out = nc.dram_tensor("out", [SQ, D], F32, kind="ExternalOutput").ap()

    with tile.TileContext(nc) as tc, ExitStack() as ctx:
        singles = ctx.enter_context(tc.tile_pool(name="singles", bufs=1))
        qkv = ctx.enter_context(tc.tile_pool(name="qkv", bufs=1))
        xp = ctx.enter_context(tc.tile_pool(name="xp", bufs=1))
        wqk = ctx.enter_context(tc.tile_pool(name="wqk", bufs=3))
        wvp = ctx.enter_context(tc.tile_pool(name="wvp", bufs=1))
        wop = ctx.enter_context(tc.tile_pool(name="wop", bufs=1))
        expp = ctx.enter_context(tc.tile_pool(name="expp", bufs=5))
        scrp = ctx.enter_context(tc.tile_pool(name="scrp", bufs=6))
        outp = ctx.enter_context(tc.tile_pool(name="outp", bufs=3))
        lnp = ctx.enter_context(tc.tile_pool(name="lnp", bufs=3))
        mmp = ctx.enter_context(tc.tile_pool(name="mm", bufs=2, space="PSUM"))
        scp = ctx.enter_context(tc.tile_pool(name="scp", bufs=2, space="PSUM"))
        avp = ctx.enter_context(tc.tile_pool(name="avp", bufs=2, space="PSUM"))

        nc.gpsimd.load_library(library_config.proxy)

        qt = qkv.tile([P, ET, SQ], dtype, tag="qt")
        kt = qkv.tile([P, ET, S], dtype, tag="kt")
        vt = qkv.tile([P, SJ, H, HD + 1], dtype, tag="vt")
        aot = qkv.tile([P, ET, SQ], dtype, tag="aot")

        def load_wqk(m):
            wqm = wqk.tile([P, ET, P], dtype, tag="wqk", name="wqm")
            nc.scalar.dma_start(wqm, ws["wq"][m].rearrange("p (t d) -> p t d", d=P))
            wkm = wqk.tile([P, ET, P], dtype, tag="wqk", name="wkm")
            nc.scalar.dma_start(wkm, ws["wk"][m].rearrange("p (t d) -> p t d", d=P))
            return wqm, wkm

        # --- load x^T per e-tile, alternating HWDGE queues so the
        # projections (which consume tiles faster than one queue delivers)
        # aren't DMA-starved at startup; first head-pair weights go first
        pre0 = load_wqk(0)
        pre1 = load_wqk(1)
        xt = xp.tile([P, ET, S], dtype, tag="xt")
        H2 = S // 2
        for k in range(ET):
            for h in range(2):
                eng = nc.sync if (2 * k + h) % 2 == 0 else nc.scalar
                eng.dma_start(
                    xt[:, k, h * H2:(h + 1) * H2],
                    xT.rearrange("(t p) s -> p t s", p=P)[:, k, h * H2:(h + 1) * H2])

        # --- constants ---
        bqk = singles.tile([P, 2 * ET], F32, tag="bqk")
        nc.sync.dma_start(bqk[:, :ET], bs["bq"].rearrange("(t p) -> p t", p=P))
        nc.sync.dma_start(bqk[:, ET:], bs["bk"].rearrange("(t p) -> p t", p=P))
        # free-dim bias rows, physically replicated across partitions
        # (compute engines can't read partition-step-0 APs)
        brow = {}
        for b in ("bv", "bo", "lnw", "lnb"):
            t = singles.tile([P, D], F32, tag=b)
            nc.sync.dma_start(t[0:1, :], bs[b][None, :])
            nc.gpsimd.partition_broadcast(t, t[0:1, :])
            brow[b] = t
        eps = singles.tile([P, 1], F32, tag="eps")
        nc.vector.memset(eps, 1e-5)
        nc.vector.memset(vt[:, :, :, HD:HD + 1], 1.0)
        idt = singles.tile([P, P], dtype, tag="idt")
        nc.sync.dma_start(idt, ident)
        zrow = singles.tile([P, 512], dtype, tag="zrow")
        nc.vector.memset(zrow, 0.0)

        def qk_proj(m, pre=None):
            """q and k projections for head-pair m (d rows m*128..)."""
            wqm, wkm = pre if pre is not None else load_wqk(m)
            for n in range(QN):
                ps = mmp.tile([P, 512], F32, tag="mm", name="ps")[:, :QC]
                for k in range(ET):
                    nc.tensor.matmul(
                        ps, wqm[:, k, :], xt[:, k, n * QC:(n + 1) * QC],
                        start=(k == 0), stop=(k == ET - 1),
                    )
                nc.vector.tensor_scalar_add(
                    qt[:, m, n * QC:(n + 1) * QC], ps, bqk[:, m:m + 1])
            for n in range(KN):
                ps = mmp.tile([P, 512], F32, tag="mm", name="ps")[:, :KC]
                for k in range(ET):
                    nc.tensor.matmul(
                        ps, wkm[:, k, :], xt[:, k, n * KC:(n + 1) * KC],
                        start=(k == 0), stop=(k == ET - 1),
                    )
                nc.vector.tensor_scalar_add(
                    kt[:, m, n * KC:(n + 1) * KC], ps,
                    bqk[:, ET + m:ET + m + 1])

        def load_wv(n):
            wvn = wvp.tile([P, ET, VC], dtype, tag="wv", name="wvn")
            nc.scalar.dma_start(
                wvn,
                ws["wv"].rearrange("(t p) d -> p t d", p=P)[:, :, n * VC:(n + 1) * VC])
            return wvn

        def v_block(n, wvn, j0):
            """v projection d-chunk n, s-tiles j0..j0+1."""
            for j in range(j0, min(j0 + 2, SJ)):
                ps = mmp.tile([P, 512], F32, tag="mm", name="ps")[:, :VC]
                for k in range(ET):
                    nc.tensor.matmul(
                        ps, xt[:, k, j * P:(j + 1) * P], wvn[:, k, :],
                        start=(k == 0), stop=(k == ET - 1),
                    )
                nc.vector.tensor_tensor(
                    vt[:, j, n * HPC:(n + 1) * HPC, 0:HD],
                    ps.rearrange("p (h d) -> p h d", d=HD),
                    brow["bv"][:, n * VC:(n + 1) * VC].rearrange(
                        "p (h d) -> p h d", d=HD),
                    ALU.add,
                )

        def v_proj(n, wvn):
            for j0 in range(0, SJ, 2):
                v_block(n, wvn, j0)

        # Schraudolph exp for the DVE-offloaded score tiles: the bf16 bit
        # pattern of exp(s/8) is approximated by int16(s*A + B); the linear
        # mantissa interpolation costs ~1.8% rms on those keys, which the
        # softmax ratio mostly cancels.
        SCH_A = 0.125 * 128.0 / float(np.log(2.0))
        SCH_B = 16256.0 - 7.25

        def att_exp(m, n, j, on_dve=False):
            """score pair + exp for (head pair m, q-chunk n, k-tile j)."""
            sc = scp.tile([P, 2, 512], F32, tag="sc", name="sc")
            nc.tensor.matmul(
                sc[:, 0, :QC],
                kt[0:HD, m, j * P:(j + 1) * P],
                qt[0:HD, m, n * QC:(n + 1) * QC],
            )
            nc.tensor.matmul(
                sc[:, 1, :QC],
                kt[HD:P, m, j * P:(j + 1) * P],
                qt[HD:P, m, n * QC:(n + 1) * QC],
            )
            et = expp.tile([P, 2, 512], dtype, tag="exp", name="et")
            if on_dve:
                nc.vector.tensor_scalar(
                    et.bitcast(mybir.dt.int16)[:, :, :QC], sc[:, :, :QC],
                    SCH_A, SCH_B, ALU.mult, ALU.add)
            else:
                nc.scalar.activation(et[:, :, :QC], sc[:, :, :QC], AF.Exp,
                                     scale=0.125)
            return et

        QT = QC // P  # 128-row q subtiles per q-chunk

        def attention(m, n, deferred):
            """q-chunk n of head pair m (heads 2m, 2m+1). AV-swap: et is
            stationary, v_aug moves; accumulators av_h[:, t, 0:65] hold
            [q, av|denom] per q-subtile t, one PSUM bank per head.

            `deferred` holds the previous block's transpose emitters; they
            are woven into this block's j-loop so the in-order PE stream
            never stalls on the previous block's DVE norms at the block
            boundary. Returns this block's deferred emitters."""
            avs = [None, None]

            def emit_avs(et, j):
                for h in range(2):
                    if avs[h] is None:
                        # lazy: the zero matmul (clears the bank's
                        # has_written bits and orders, via the full-tile
                        # write, ahead of the start=False accumulation)
                        # waits on the previous block's norm reads through
                        # pool-buf reuse, so it must come after this
                        # block's first scores in the stream
                        avs[h] = avp.tile([P, QT, P], F32, tag="av",
                                          name="av")
                        nc.tensor.matmul(
                            avs[h].rearrange("p t d -> p (t d)"), idt, zrow,
                            start=True, stop=True, skip_group_check=True)
                    av = avs[h]
                    for t in range(QT):
                        nc.tensor.matmul(
                            av[:, t, :HD + 1],
                            et[:, h, t * P:(t + 1) * P],
                            vt[:, j, 2 * m + h, :],
                            start=False, stop=(j == SJ - 1 and t == QT - 1),
                            skip_group_check=True,
                        )

            # software pipeline: av(j-1) is emitted after score/exp(j) so the
            # in-order PE stream never stalls a score matmul behind an av
            # that waits on exp(j)
            prev = None
            for j in range(SJ):
                et = att_exp(m, n, j, on_dve=(j % 4 == 2))
                if deferred and j in (2, 4, 6, 8):
                    deferred.pop(0)()
                if prev is not None:
                    emit_avs(prev, j - 1)
                prev = et
            emit_avs(prev, SJ - 1)
            for fn in deferred:
                fn()
            scr = scrp.tile([P, QT, P], dtype, tag="scr", name="scr")
            for h, av in enumerate(avs):
                rcp = scrp.tile([P, QT], F32, tag="rcp", name="rcp")
                nc.vector.reciprocal(rcp, av[:, :, HD])
                for t in range(QT):
                    nc.vector.tensor_scalar_mul(
                        scr[:, t, h * HD:(h + 1) * HD],
                        av[:, t, :HD], rcp[:, t:t + 1])

            def transp(t):
                def emit():
                    tp = mmp.tile([P, P], dtype, tag="mm", name="tp")
                    nc.tensor.transpose(tp, scr[:, t, :], idt)
                    nc.vector.tensor_copy(
                        aot[:, m, n * QC + t * P:n * QC + (t + 1) * P], tp)
                return emit

            return [transp(t) for t in range(QT)]

        def out_ln(t):
            """Out-projection + LayerNorm for q row tile t."""
            FSUB = min(512, D)
            NSUB = D // FSUB
            ot = outp.tile([P, D], F32, tag="ot", name="ot")
            for nn in range(VN):
                ps = mmp.tile([P, 512], F32, tag="mm", name="ps")[:, :VC]
                for k in range(ET):
                    nc.tensor.matmul(
                        ps, aot[:, k, t * P:(t + 1) * P],
                        wo[:, k, nn * VC:(nn + 1) * VC],
                        start=(k == 0), stop=(k == ET - 1),
                    )
                nc.vector.tensor_tensor(
                    ot[:, nn * VC:(nn + 1) * VC], ps,
                    brow["bo"][:, nn * VC:(nn + 1) * VC], ALU.add)
            scr = lnp.tile([P, NSUB * 6 + 3], F32, tag="scr", name="scr")
            stats = scr[:, 0:NSUB * 6].rearrange("p (s f) -> p s f", f=6)
            mv = scr[:, NSUB * 6:NSUB * 6 + 2]
            rstd = scr[:, NSUB * 6 + 2:NSUB * 6 + 3]
            otv = ot.rearrange("p (s f) -> p s f", f=FSUB)
            for sbi in range(NSUB):
                nc.vector.bn_stats(stats[:, sbi, :], otv[:, sbi, :])
            nc.vector.bn_aggr(mv, stats)
            nc.scalar.activation(rstd, mv[:, 1:2], AF.Sqrt, bias=eps)
            nc.vector.reciprocal(rstd, rstd)
            nc.vector.tensor_scalar(
                ot, ot, mv[:, 0:1], rstd, ALU.subtract, ALU.mult)
            # scale/shift on GpSimd: it's idle in the tail while DVE is
            # the critical path for the LN chains and PSUM epilogues
            nc.gpsimd.tensor_tensor(ot, ot, brow["lnw"], ALU.mult)
            nc.gpsimd.tensor_tensor(ot, ot, brow["lnb"], ALU.add)
            nc.sync.dma_start(
                out.rearrange("(t p) d -> p t d", p=P)[:, t, :], ot)

        # --- emission schedule ---
        # q/k for the first v-chunk's head pairs, then v chunk 0, then
        # alternate attention blocks with the remaining projections so the
        # scheduler can fill PE gaps under ScalarE-bound attention.
        wo = wop.tile([P, ET, D], dtype, tag="wo")
        qk_proj(0, pre0)
        qk_proj(1, pre1)
        wv0 = load_wv(0)
        v_proj(0, wv0)
        nc.scalar.dma_start(wo, ws["wo"].rearrange("(t p) d -> p t d", p=P))
        # n-outer: the n=0 sweep over all head pairs interleaves with the
        # remaining q/k/v projections; after it, aot columns [0, QC) are
        # complete, so out_ln(0..QT-1) becomes the PE filler for the
        # otherwise projection-dry n=1 sweep.
        deferred = []
        for n in range(QN):
            for m in range(ET):
                deferred = attention(m, n, deferred)
                if n == 0:
                    if 2 <= m + 1 < ET and m + 1 != 1:
                        qk_proj(m + 1)
                    for vn in range(1, VN):
                        if m + 2 == vn * MPC:
                            v_proj(vn, load_wv(vn))
                elif m % 2 == 1:
                    t = n * QT + (m - 1) // 2
                    if t < TQ:
                        out_ln_t = (m - 1) // 2
                        out_ln(out_ln_t)
        for fn in deferred:
            fn()
        # tail: out-proj/LN for the last q-chunk's row tiles
        for t in range(QT, TQ):
            out_ln(t)

    nc.compile()
    return nc


# ---------------------------------------------------------------- host side

_CACHE = {}


def _get_nc(S, SQ, D, H):
    key = (S, SQ, D, H)
    if key not in _CACHE:
        _CACHE[key] = build_bass(S, SQ, D, H)
    return _CACHE[key]


def make_in_maps(x, Wq, bq, Wk, bk, Wv, bv, Wo, bo, ln_w, ln_b, n_cores=8):
    """Shard full inputs into per-core input maps (batch x seq-half)."""
    B, S, D = x.shape
    halves = n_cores // B
    SQ = S // halves
    bf = ml_dtypes.bfloat16
    ET = D // 128

    def pack_qk(W):
        # [m, p, t*128+d] = W.T[t*128+p, m*128+d]
        w4 = np.asarray(W).T.reshape(ET, 128, ET, 128)  # [t, p, m, d]
        return np.ascontiguousarray(
            w4.transpose(2, 1, 0, 3).reshape(ET, 128, ET * 128)).astype(bf)

    common = {
        "ident": np.eye(128, dtype=bf),
        "wq": pack_qk(Wq),
        "wk": pack_qk(Wk),
        "wv": np.ascontiguousarray(np.asarray(Wv).T).astype(bf),
        "wo": np.ascontiguousarray(np.asarray(Wo).T).astype(bf),
        "bq": np.asarray(bq, np.float32), "bk": np.asarray(bk, np.float32),
        "bv": np.asarray(bv, np.float32), "bo": np.asarray(bo, np.float32),
        "lnw": np.asarray(ln_w, np.float32), "lnb": np.asarray(ln_b, np.float32),
    }
    in_maps = []
    for c in range(n_cores):
        b, half = c // halves, c % halves
        xTb = np.asarray(x[b]).T.astype(bf)  # [D, S]
        if half:
            xTb = np.roll(xTb, -half * SQ, axis=1)
        in_maps.append({"xT": np.ascontiguousarray(xTb), **common})
    return in_maps, SQ


def kernel(x, Wq, bq, Wk, bk, Wv, bv, Wo, bo, ln_w, ln_b, _trace=False):
    x = np.asarray(x)
    B, S, D = x.shape
    n_cores = 8
    in_maps, SQ = make_in_maps(x, Wq, bq, Wk, bk, Wv, bv, Wo, bo, ln_w, ln_b,
                               n_cores)
    nc = _get_nc(S, SQ, D, 16)
    res = run_bass_kernel_spmd(nc, in_maps, list(range(n_cores)), trace=_trace)
    out = np.empty((B, S, D), np.float32)
    halves = n_cores // B
    for c in range(n_cores):
        b, half = c // halves, c % halves
        out[b, half * SQ:(half + 1) * SQ] = res.results[c]["out"]
    kernel.last_result = res
    return out


if __name__ == "__main__":
    nc = build_bass(512, 256, 256, 4)
    print("built ok")



# revision 21
# speedup vs baseline: 1.0021x; 1.0021x over previous
"""Trainium2 Bass kernel for MultiHeadAttention (B=4, S=2048, D=1024, H=16).

Sharding: 8 cores = 4 batches x 2 sequence-halves, no collectives. Each
core computes full k/v projections for its batch and q/attention/
out-proj/LayerNorm for its half of the sequence. A host-side column
roll of x^T makes the program identical across cores (softmax over
keys is permutation-invariant, so the k/v column order doesn't
matter): the core's q rows are always columns [0, SQ) of its x^T.

Device program (per core), all matmuls bf16 with fp32 PSUM accumulate:
  qT = Wq @ xT[:, :SQ] + bq   [D, SQ]   (d on partitions)
  kT = Wk @ xT + bk           [D, S]
  v  = x @ Wv.T + bv          [S, D]    (s on partitions), augmented
                                        with a ones column per head
  attention per head-pair m (heads 2m / 2m+1 live on partition halves
  0:64 / 64:128 of kT/qT tile m), per q-chunk n:
    scT[j]   = kT_h[:, j-chunk] . qT_h     both heads -> one 2-bank
                                           PSUM tile [128, 2, 512]
    eT[j]    = exp(scT[j] / 8)             one wide ScalarE op, bf16
    av_h[t] += eT_h[j][:, t-chunk].T @ v_aug_h[j]
        "AV-swap": eT is the stationary operand, v_aug (64 cols + ones
        column) moves, so each matmul streams only 65 columns instead
        of 512 -> half the PE time of the classic v.T @ e orientation.
        Accumulators for one head live in one PSUM bank as [4, 65]
        slices of a [128, 4, 128] tile; a full-bank zero matmul opens
        each round (clears the has_written bits + orders the start),
        then all real matmuls accumulate with start=False.
    ao_h[t]  = av_h[t][:, 0:64] * (1/av_h[t][:, 64])   per-partition
                                           scalar -> one tensor_scalar
    aot      = PE-transpose(ao)            [q, d] -> [d, q] via the
                                           identity matmul, per 128x128
  out = aoT.T @ Wo.T + bo; LayerNorm -> DRAM.

Scheduling: weights stream per head-pair so attention on pair m starts
as soon as q/k(m) and the first v d-chunk exist; the remaining
projections fill PE slack underneath the ScalarE-bound attention.
"""

import os
import sys
from contextlib import ExitStack

for _p in ("/opt/trn_rl_repo", "/root/.axon_site/_ro/trn_rl_repo"):
    if _p not in sys.path and os.path.isdir(_p):
        sys.path.insert(0, _p)

# The kernel executes through the axon jax platform; a cpu-pinned
# JAX_PLATFORMS (used for running references) would hide the NeuronCores.
# Only safe to fix before jax is first imported.
if "jax" not in sys.modules and "axon" not in os.environ.get(
        "JAX_PLATFORMS", "axon"):
    os.environ.pop("JAX_PLATFORMS")

import ml_dtypes
import numpy as np

import concourse.bacc as bacc
import concourse.mybir as mybir
import concourse.tile as tile
from concourse import library_config
from concourse.bass_utils import run_bass_kernel_spmd

BF16 = mybir.dt.bfloat16
F32 = mybir.dt.float32
AF = mybir.ActivationFunctionType
ALU = mybir.AluOpType

HD = 64  # head dim


def build_bass(S, SQ, D, H, dtype=BF16):
    """Build the per-core Bass program. S: kv seq len, SQ: q rows handled
    by this core, D: embed dim, H: total heads."""
    assert D == H * HD
    P = 128
    ET = D // P           # e (contraction) tiles; also head-pair count
    QC = min(512, SQ)     # q free-dim chunk
    QN = SQ // QC
    KC = min(512, S)      # k-proj free-dim chunk
    KN = S // KC
    VC = min(512, D)      # v/out-proj d chunk
    VN = D // VC
    HPC = VC // HD        # heads per v chunk
    MPC = HPC // 2        # head-pairs per v chunk
    SJ = S // P           # key tiles
    TQ = SQ // P          # q row tiles

    nc = bacc.Bacc("TRN2", debug=False)

    xT = nc.dram_tensor("xT", [D, S], dtype, kind="ExternalInput").ap()
    ident = nc.dram_tensor("ident", [P, P], dtype, kind="ExternalInput").ap()
    ws = {}
    for w in ("wq", "wk"):  # host-packed per head-pair: [m, p, t*d]
        ws[w] = nc.dram_tensor(w, [ET, P, ET * P], dtype,
                               kind="ExternalInput").ap()
    for w in ("wv", "wo"):
        ws[w] = nc.dram_tensor(w, [D, D], dtype, kind="ExternalInput").ap()
    bs = {
        b: nc.dram_tensor(b, [D], F32, kind="ExternalInput").ap()
        for b in ("bq", "bk", "bv", "bo", "lnw", "lnb")
    }
    out = nc.dram_tensor("out", [SQ, D], F32, kind="ExternalOutput").ap()

    with tile.TileContext(nc) as tc, ExitStack() as ctx:
        singles = ctx.enter_context(tc.tile_pool(name="singles", bufs=1))
        qkv = ctx.enter_context(tc.tile_pool(name="qkv", bufs=1))
        xp = ctx.enter_context(tc.tile_pool(name="xp", bufs=1))
        wqk = ctx.enter_context(tc.tile_pool(name="wqk", bufs=3))
        wvp = ctx.enter_context(tc.tile_pool(name="wvp", bufs=1))
        wop = ctx.enter_context(tc.tile_pool(name="wop", bufs=1))
        expp = ctx.enter_context(tc.tile_pool(name="expp", bufs=5))
        scrp = ctx.enter_context(tc.tile_pool(name="scrp", bufs=6))
        outp = ctx.enter_context(tc.tile_pool(name="outp", bufs=3))
        lnp = ctx.enter_context(tc.tile_pool(name="lnp", bufs=3))
        mmp = ctx.enter_context(tc.tile_pool(name="mm", bufs=2, space="PSUM"))
        scp = ctx.enter_context(tc.tile_pool(name="scp", bufs=2, space="PSUM"))
        avp = ctx.enter_context(tc.tile_pool(name="avp", bufs=2, space="PSUM"))

        nc.gpsimd.load_library(library_config.proxy)

        qt = qkv.tile([P, ET, SQ], dtype, tag="qt")
        kt = qkv.tile([P, ET, S], dtype, tag="kt")
        vt = qkv.tile([P, SJ, H, HD + 1], dtype, tag="vt")
        aot = qkv.tile([P, ET, SQ], dtype, tag="aot")

        def load_wqk(m):
            wqm = wqk.tile([P, ET, P], dtype, tag="wqk", name="wqm")
            nc.scalar.dma_start(wqm, ws["wq"][m].rearrange("p (t d) -> p t d", d=P))
            wkm = wqk.tile([P, ET, P], dtype, tag="wqk", name="wkm")
            nc.scalar.dma_start(wkm, ws["wk"][m].rearrange("p (t d) -> p t d", d=P))
            return wqm, wkm

        # --- load x^T per e-tile, alternating HWDGE queues so the
        # projections (which consume tiles faster than one queue delivers)
        # aren't DMA-starved at startup; first head-pair weights go first
        pre0 = load_wqk(0)
        pre1 = load_wqk(1)
        xt = xp.tile([P, ET, S], dtype, tag="xt")
        H2 = S // 2
        # h-outer: everything the first half of the kernel touches (q n=0,
        # k chunks 0-1, v/attention j-tiles 0..SJ/2) lives in sequence half
        # 0, so land all of half 0 first, split across both queues
        for h in range(2):
            for k in range(ET):
                eng = nc.sync if k % 2 == 0 else nc.scalar
                eng.dma_start(
                    xt[:, k, h * H2:(h + 1) * H2],
                    xT.rearrange("(t p) s -> p t s", p=P)[:, k, h * H2:(h + 1) * H2])

        # --- constants ---
        bqk = singles.tile([P, 2 * ET], F32, tag="bqk")
        nc.sync.dma_start(bqk[:, :ET], bs["bq"].rearrange("(t p) -> p t", p=P))
        nc.sync.dma_start(bqk[:, ET:], bs["bk"].rearrange("(t p) -> p t", p=P))
        # free-dim bias rows, physically replicated across partitions
        # (compute engines can't read partition-step-0 APs)
        brow = {}
        for b in ("bv", "bo", "lnw", "lnb"):
            t = singles.tile([P, D], F32, tag=b)
            nc.sync.dma_start(t[0:1, :], bs[b][None, :])
            nc.gpsimd.partition_broadcast(t, t[0:1, :])
            brow[b] = t
        eps = singles.tile([P, 1], F32, tag="eps")
        nc.vector.memset(eps, 1e-5)
        nc.vector.memset(vt[:, :, :, HD:HD + 1], 1.0)
        idt = singles.tile([P, P], dtype, tag="idt")
        nc.sync.dma_start(idt, ident)
        zrow = singles.tile([P, 512], dtype, tag="zrow")
        nc.vector.memset(zrow, 0.0)

        def qk_proj(m, pre=None):
            """q and k projections for head-pair m (d rows m*128..)."""
            wqm, wkm = pre if pre is not None else load_wqk(m)
            for n in range(QN):
                ps = mmp.tile([P, 512], F32, tag="mm", name="ps")[:, :QC]
                for k in range(ET):
                    nc.tensor.matmul(
                        ps, wqm[:, k, :], xt[:, k, n * QC:(n + 1) * QC],
                        start=(k == 0), stop=(k == ET - 1),
                    )
                nc.vector.tensor_scalar_add(
                    qt[:, m, n * QC:(n + 1) * QC], ps, bqk[:, m:m + 1])
            for n in range(KN):
                ps = mmp.tile([P, 512], F32, tag="mm", name="ps")[:, :KC]
                for k in range(ET):
                    nc.tensor.matmul(
                        ps, wkm[:, k, :], xt[:, k, n * KC:(n + 1) * KC],
                        start=(k == 0), stop=(k == ET - 1),
                    )
                nc.vector.tensor_scalar_add(
                    kt[:, m, n * KC:(n + 1) * KC], ps,
                    bqk[:, ET + m:ET + m + 1])

        def load_wv(n):
            wvn = wvp.tile([P, ET, VC], dtype, tag="wv", name="wvn")
            nc.scalar.dma_start(
                wvn,
                ws["wv"].rearrange("(t p) d -> p t d", p=P)[:, :, n * VC:(n + 1) * VC])
            return wvn

        def v_block(n, wvn, j0):
            """v projection d-chunk n, s-tiles j0..j0+1."""
            for j in range(j0, min(j0 + 2, SJ)):
                ps = mmp.tile([P, 512], F32, tag="mm", name="ps")[:, :VC]
                for k in range(ET):
                    nc.tensor.matmul(
                        ps, xt[:, k, j * P:(j + 1) * P], wvn[:, k, :],
                        start=(k == 0), stop=(k == ET - 1),
                    )
                nc.vector.tensor_tensor(
                    vt[:, j, n * HPC:(n + 1) * HPC, 0:HD],
                    ps.rearrange("p (h d) -> p h d", d=HD),
                    brow["bv"][:, n * VC:(n + 1) * VC].rearrange(
                        "p (h d) -> p h d", d=HD),
                    ALU.add,
                )

        def v_proj(n, wvn):
            for j0 in range(0, SJ, 2):
                v_block(n, wvn, j0)

        # Schraudolph exp for the DVE-offloaded score tiles: the bf16 bit
        # pattern of exp(s/8) is approximated by int16(s*A + B); the linear
        # mantissa interpolation costs ~1.8% rms on those keys, which the
        # softmax ratio mostly cancels.
        SCH_A = 0.125 * 128.0 / float(np.log(2.0))
        SCH_B = 16256.0 - 7.25

        def dve_exp(n, j):
            # the n=0 sweep is PE-bound (projection filler available), so
            # ScalarE alone keeps up; the filler-dry n=1 sweep needs the
            # exp cadence split across both engines
            return j % 2 == 1 if n == 1 else False

        def att_exp(m, n, j, on_dve=False):
            """score pair + exp for (head pair m, q-chunk n, k-tile j)."""
            sc = scp.tile([P, 2, 512], F32, tag="sc", name="sc")
            nc.tensor.matmul(
                sc[:, 0, :QC],
                kt[0:HD, m, j * P:(j + 1) * P],
                qt[0:HD, m, n * QC:(n + 1) * QC],
            )
            nc.tensor.matmul(
                sc[:, 1, :QC],
                kt[HD:P, m, j * P:(j + 1) * P],
                qt[HD:P, m, n * QC:(n + 1) * QC],
            )
            et = expp.tile([P, 2, 512], dtype, tag="exp", name="et")
            if on_dve:
                nc.vector.tensor_scalar(
                    et.bitcast(mybir.dt.int16)[:, :, :QC], sc[:, :, :QC],
                    SCH_A, SCH_B, ALU.mult, ALU.add)
            else:
                nc.scalar.activation(et[:, :, :QC], sc[:, :, :QC], AF.Exp,
                                     scale=0.125)
            return et

        QT = QC // P  # 128-row q subtiles per q-chunk

        def attention(m, n, deferred):
            """q-chunk n of head pair m (heads 2m, 2m+1). AV-swap: et is
            stationary, v_aug moves; accumulators av_h[:, t, 0:65] hold
            [q, av|denom] per q-subtile t, one PSUM bank per head.

            `deferred` holds the previous block's transpose emitters; they
            are woven into this block's j-loop so the in-order PE stream
            never stalls on the previous block's DVE norms at the block
            boundary. Returns this block's deferred emitters."""
            avs = [None, None]

            def emit_avs(et, j):
                for h in range(2):
                    if avs[h] is None:
                        # lazy: the zero matmul (clears the bank's
                        # has_written bits and orders, via the full-tile
                        # write, ahead of the start=False accumulation)
                        # waits on the previous block's norm reads through
                        # pool-buf reuse, so it must come after this
                        # block's first scores in the stream
                        avs[h] = avp.tile([P, QT, P], F32, tag="av",
                                          name="av")
                        # 4-element zero matmul: start=True clears the whole
                        # bank's has_written bits; writing one column of each
                        # accumulator slice orders it ahead of all 4 groups
                        nc.tensor.matmul(
                            avs[h][:, :, 0], idt, zrow[:, :QT],
                            start=True, stop=True, skip_group_check=True)
                    av = avs[h]
                    for t in range(QT):
                        nc.tensor.matmul(
                            av[:, t, :HD + 1],
                            et[:, h, t * P:(t + 1) * P],
                            vt[:, j, 2 * m + h, :],
                            start=False, stop=(j == SJ - 1 and t == QT - 1),
                            skip_group_check=True,
                        )

            # software pipeline: av(j-1) is emitted after score/exp(j) so the
            # in-order PE stream never stalls a score matmul behind an av
            # that waits on exp(j)
            prev = None
            for j in range(SJ):
                et = att_exp(m, n, j, on_dve=dve_exp(n, j))
                if deferred and j in (2, 4, 6, 8):
                    deferred.pop(0)()
                if prev is not None:
                    emit_avs(prev, j - 1)
                prev = et
            emit_avs(prev, SJ - 1)
            for fn in deferred:
                fn()
            scr = scrp.tile([P, QT, P], dtype, tag="scr", name="scr")
            for h, av in enumerate(avs):
                rcp = scrp.tile([P, QT], F32, tag="rcp", name="rcp")
                nc.vector.reciprocal(rcp, av[:, :, HD])
                for t in range(QT):
                    nc.vector.tensor_scalar_mul(
                        scr[:, t, h * HD:(h + 1) * HD],
                        av[:, t, :HD], rcp[:, t:t + 1])

            def transp(t):
                def emit():
                    tp = mmp.tile([P, P], dtype, tag="mm", name="tp")
                    nc.tensor.transpose(tp, scr[:, t, :], idt)
                    nc.vector.tensor_copy(
                        aot[:, m, n * QC + t * P:n * QC + (t + 1) * P], tp)
                return emit

            return [transp(t) for t in range(QT)]

        def out_ln(t, tail=False):
            """Out-projection + LayerNorm for q row tile t."""
            FSUB = min(512, D)
            NSUB = D // FSUB
            ot = outp.tile([P, D], F32, tag="ot", name="ot")
            for nn in range(VN):
                ps = mmp.tile([P, 512], F32, tag="mm", name="ps")[:, :VC]
                for k in range(ET):
                    nc.tensor.matmul(
                        ps, aot[:, k, t * P:(t + 1) * P],
                        wo[:, k, nn * VC:(nn + 1) * VC],
                        start=(k == 0), stop=(k == ET - 1),
                    )
                nc.vector.tensor_tensor(
                    ot[:, nn * VC:(nn + 1) * VC], ps,
                    brow["bo"][:, nn * VC:(nn + 1) * VC], ALU.add)
            scr = lnp.tile([P, NSUB * 6 + 3], F32, tag="scr", name="scr")
            stats = scr[:, 0:NSUB * 6].rearrange("p (s f) -> p s f", f=6)
            mv = scr[:, NSUB * 6:NSUB * 6 + 2]
            rstd = scr[:, NSUB * 6 + 2:NSUB * 6 + 3]
            otv = ot.rearrange("p (s f) -> p s f", f=FSUB)
            for sbi in range(NSUB):
                nc.vector.bn_stats(stats[:, sbi, :], otv[:, sbi, :])
            nc.vector.bn_aggr(mv, stats)
            nc.scalar.activation(rstd, mv[:, 1:2], AF.Sqrt, bias=eps)
            nc.vector.reciprocal(rstd, rstd)
            nc.vector.tensor_scalar(
                ot, ot, mv[:, 0:1], rstd, ALU.subtract, ALU.mult)
            # scale/shift on GpSimd mid-kernel (it idles there); the final
            # tiles go through DVE, whose op is ~2x faster, to shorten the
            # post-attention serial chain that sets the kernel end time
            eng = nc.vector if tail else nc.gpsimd
            eng.tensor_tensor(ot, ot, brow["lnw"], ALU.mult)
            eng.tensor_tensor(ot, ot, brow["lnb"], ALU.add)
            nc.sync.dma_start(
                out.rearrange("(t p) d -> p t d", p=P)[:, t, :], ot)

        # --- emission schedule ---
        # q/k for the first v-chunk's head pairs, then v chunk 0, then
        # alternate attention blocks with the remaining projections so the
        # scheduler can fill PE gaps under ScalarE-bound attention.
        wo = wop.tile([P, ET, D], dtype, tag="wo")
        qk_proj(0, pre0)
        qk_proj(1, pre1)
        wv0 = load_wv(0)
        v_proj(0, wv0)
        nc.scalar.dma_start(wo, ws["wo"].rearrange("(t p) d -> p t d", p=P))
        # n-outer: the n=0 sweep over all head pairs interleaves with the
        # remaining q/k/v projections; after it, aot columns [0, QC) are
        # complete, so out_ln(0..QT-1) becomes the PE filler for the
        # otherwise projection-dry n=1 sweep.
        deferred = []
        for n in range(QN):
            for m in range(ET):
                deferred = attention(m, n, deferred)
                if n == 0:
                    if 2 <= m + 1 < ET and m + 1 != 1:
                        qk_proj(m + 1)
                    for vn in range(1, VN):
                        if m + 2 == vn * MPC:
                            v_proj(vn, load_wv(vn))
                elif m % 2 == 1:
                    t = n * QT + (m - 1) // 2
                    if t < TQ:
                        out_ln_t = (m - 1) // 2
                        out_ln(out_ln_t)
        for fn in deferred:
            fn()
        # tail: out-proj/LN for the last q-chunk's row tiles
        for t in range(QT, TQ):
            out_ln(t, tail=True)

    nc.compile()
    return nc


# ---------------------------------------------------------------- host side

_CACHE = {}


def _get_nc(S, SQ, D, H):
    key = (S, SQ, D, H)
    if key not in _CACHE:
        _CACHE[key] = build_bass(S, SQ, D, H)
    return _CACHE[key]


def make_in_maps(x, Wq, bq, Wk, bk, Wv, bv, Wo, bo, ln_w, ln_b, n_cores=8):
    """Shard full inputs into per-core input maps (batch x seq-half)."""
    B, S, D = x.shape
    halves = n_cores // B
    SQ = S // halves
    bf = ml_dtypes.bfloat16
    ET = D // 128

    def pack_qk(W):
        # [m, p, t*128+d] = W.T[t*128+p, m*128+d]
        w4 = np.asarray(W).T.reshape(ET, 128, ET, 128)  # [t, p, m, d]
        return np.ascontiguousarray(
            w4.transpose(2, 1, 0, 3).reshape(ET, 128, ET * 128)).astype(bf)

    common = {
        "ident": np.eye(128, dtype=bf),
        "wq": pack_qk(Wq),
        "wk": pack_qk(Wk),
        "wv": np.ascontiguousarray(np.asarray(Wv).T).astype(bf),
        "wo": np.ascontiguousarray(np.asarray(Wo).T).astype(bf),
        "bq": np.asarray(bq, np.float32), "bk": np.asarray(bk, np.float32),
        "bv": np.asarray(bv, np.float32), "bo": np.asarray(bo, np.float32),
        "lnw": np.asarray(ln_w, np.float32), "lnb": np.asarray(ln_b, np.float32),
    }
    in_maps = []
    for c in range(n_cores):
        b, half = c // halves, c % halves
        xTb = np.asarray(x[b]).T.astype(bf)  # [D, S]
        if half:
            xTb = np.roll(xTb, -half * SQ, axis=1)
        in_maps.append({"xT": np.ascontiguousarray(xTb), **common})
    return in_maps, SQ


def kernel(x, Wq, bq, Wk, bk, Wv, bv, Wo, bo, ln_w, ln_b, _trace=False):
    x = np.asarray(x)
    B, S, D = x.shape
    n_cores = 8
    in_maps, SQ = make_in_maps(x, Wq, bq, Wk, bk, Wv, bv, Wo, bo, ln_w, ln_b,
                               n_cores)
    nc = _get_nc(S, SQ, D, 16)
    res = run_bass_kernel_spmd(nc, in_maps, list(range(n_cores)), trace=_trace)
    out = np.empty((B, S, D), np.float32)
    halves = n_cores // B
    for c in range(n_cores):
        b, half = c // halves, c % halves
        out[b, half * SQ:(half + 1) * SQ] = res.results[c]["out"]
    kernel.last_result = res
    return out


if __name__ == "__main__":
    nc = build_bass(512, 256, 256, 4)
    print("built ok")

